# revision 12
# baseline (speedup 1.0000x reference)
"""Trainium2 Bass kernel for nn_Attention_5815385719367 (gnn_message_passing).

Computation (see reference):
  map_q/k/v = map_code @ Wq/Wk/Wv ; obs_k/v = obs_code @ Wk/Wv
  scores    = [sum(q*k,-1) | q @ obs_k.T] / 8
  w         = softmax(scores)
  agg       = w[:, :1]*glu(map_v) + w[:, 1:] @ glu(obs_v)
  out       = LN(agg @ Wo + bo + map_code) * gamma + beta

Sharding: data-parallel over N_map rows (2048 rows/core x 8 cores);
obs_code and weights replicated. No collectives.

Device kernel layout notes (per core):
  - everything streamed on-chip; the [2048, 8192] score matrix never
    touches HBM. Scores are computed TRANSPOSED: ST[obs_block=128,
    map_group=1024] = obs_kT_block.T @ qT, exp'd on ACT into PT, then
    PV matmul with gated_obs (row-major, +ones col) as the stationary
    operand accumulates aggT[33, 512] = [numer.T ; expsum] in PSUM
    over all 64 obs blocks.
  - softmax uses no max subtraction: logits are ~N(0,1), |logit| < ~6,
    exp is safe in fp32 and matches softmax exactly after dividing by
    the sum (shift invariance).
  - self-attention term handled separately: selfexp = exp(sum(q*k)/8)
    and glu(map_v) are folded in after the Wo matmul in row-major
    space (out += selfexp * (glu(map_v) @ Wo); denom += selfexp).
  - epilogue: Wo applied with the PSUM aggT as stationary operand
    ([34, 128] slabs, extended with denom/selfexp columns via an
    extended Wo), then fused DVE ops + bn_stats for LN; rsqrt via
    Newton iterations on DVE (keeps ACT on one table set: exp+tanh).
  - all matmuls use float32r (1-pass FP22 multiply, fp32 accumulate).
  - host passes map/obs pre-transposed copies so no on-device
    transposes are needed (layout prep only; all FLOPs on device).
"""

import numpy as np

import concourse.bass as bass
import concourse.bacc as bacc
import concourse.tile as tile
from concourse import mybir
from concourse.bass_utils import run_bass_kernel_spmd

NCORES = 8
NM, NO, E = 16384, 8192, 64
NS = NM // NCORES            # 2048 map rows per core
H = E // 2                   # 32
TEMP = 8.0
EPS = 1e-6
P = 128
NT = NS // P                 # 16 row tiles per core
GW = 512                     # map group width (psum bank)
NG = NS // GW                # 4 map groups
NOB = NO // P                # 64 obs blocks

F32 = mybir.dt.float32
F32R = mybir.dt.float32r
AF = mybir.ActivationFunctionType
ALU = mybir.AluOpType


def _bc_part(ap, n):
    """Broadcast a [x, ...] AP along a new leading partition dim of n."""
    return bass.AP(tensor=ap.tensor, offset=ap.offset, ap=[[0, n]] + list(ap.ap))


def _emit(tc, out_d, map_rows_d, mapT_d, obsT_d, wq_d, wk_d, wv_d, woe_d,
          bo_d, gamma_d, beta_d, ones_d, dbg=None):
    nc = tc.nc
    with tc.tile_pool(name="consts", bufs=1) as consts, \
         tc.tile_pool(name="big", bufs=1) as big, \
         tc.tile_pool(name="sb_sm", bufs=3) as sb_sm, \
         tc.tile_pool(name="sb_pt", bufs=3) as sb_pt, \
         tc.tile_pool(name="ps_aux", bufs=2, space="PSUM") as ps_aux, \
         tc.tile_pool(name="ps_st", bufs=2, space="PSUM") as ps_st, \
         tc.tile_pool(name="ps_agg", bufs=2, space="PSUM") as ps_agg:

        # ---------------- constants ----------------
        wq = consts.tile([E, E], F32R)
        nc.sync.dma_start(wq, wq_d)
        wk = consts.tile([E, E], F32R)
        nc.sync.dma_start(wk, wk_d)
        wv = consts.tile([E, E], F32R)
        nc.sync.dma_start(wv, wv_d)
        woe = consts.tile([H + 1, E + 2], F32R)
        nc.sync.dma_start(woe, woe_d)

        bo_b = consts.tile([P, E], F32)
        nc.sync.dma_start(bo_b, _bc_part(bo_d, P))
        ga_b = consts.tile([P, E], F32)
        nc.sync.dma_start(ga_b, _bc_part(gamma_d, P))
        be_b = consts.tile([P, E], F32)
        nc.sync.dma_start(be_b, _bc_part(beta_d, P))
        ones64 = consts.tile([E, 1], F32R)
        nc.sync.dma_start(ones64, ones_d.rearrange("(a b) -> a b", b=1))

        # ---------------- big arenas ----------------
        map_rows = big.tile([P, NT, E], F32)
        nc.sync.dma_start(map_rows, map_rows_d.rearrange("(t p) e -> p t e", p=P))
        mapT = big.tile([E, NS], F32R)
        nc.sync.dma_start(mapT, mapT_d)
        obsT = big.tile([E, NO], F32R)
        # chunked so prologue compute can start before the full 2MB lands
        for c in range(4):
            nc.sync.dma_start(obsT[:, c * (NO // 4):(c + 1) * (NO // 4)],
                              obsT_d[:, c * (NO // 4):(c + 1) * (NO // 4)])

        qT = big.tile([E, NS], F32R)          # map_q.T
        gmT = big.tile([H + 1, NS], F32R)     # [glu(map_v).T ; selfexp]
        okT = big.tile([E, NO], F32R)         # obs_k.T
        gob = big.tile([P, NOB, H + 1], F32R)  # row-major glu(obs_v) | ones
        ags = big.tile([H + 1, NS], F32R)     # [numer.T ; denom]
        map_pb = big.tile([P, NT, E], F32)    # map + bo
        out_pre = big.tile([P, NT, E], F32)
        out_all = big.tile([P, NT, E], F32)
        mvC = big.tile([P, NT, 2], F32)       # LN (mean, var) per tile
        rstd = big.tile([P, NT], F32)

        ones_rep = bass.AP(tensor=ones_d.tensor, offset=ones_d.offset,
                           ap=[[0, P], list(ones_d.ap[0]), [0, 1]])
        nc.sync.dma_start(gob[:, :, H:H + 1], ones_rep)
        # map_pb = map_rows + bo (bo broadcast along tile dim)
        bo_rep = bass.AP(tensor=bo_b.tensor, offset=bo_b.offset,
                         ap=[list(bo_b.ap[0]), [0, NT], list(bo_b.ap[1])])
        nc.vector.tensor_tensor(out=map_pb, in0=map_rows, in1=bo_rep,
                                op=ALU.add)

        # ---------------- prologue: map-side projections ----------------
        for c in range(NG):
            sl = slice(c * GW, (c + 1) * GW)
            q_ps = ps_aux.tile([E, GW], F32, tag="x", name=f"qps{c}")
            nc.tensor.matmul(q_ps, wq, mapT[:, sl], start=True, stop=True)
            nc.vector.tensor_copy(qT[:, sl], q_ps)

            k_ps = ps_aux.tile([E, GW], F32, tag="x", name=f"kps{c}")
            nc.tensor.matmul(k_ps, wk, mapT[:, sl], start=True, stop=True)
            qk = sb_sm.tile([E, GW], F32R, tag="qk", name=f"qk{c}")
            nc.vector.tensor_tensor(out=qk, in0=qT[:, sl], in1=k_ps, op=ALU.mult)
            ss_ps = ps_aux.tile([1, GW], F32, tag="x", name=f"ssps{c}")
            nc.tensor.matmul(ss_ps, ones64, qk, start=True, stop=True)
            nc.scalar.activation(gmT[H:H + 1, sl], ss_ps, AF.Exp,
                                 scale=1.0 / TEMP)

            v_ps = ps_aux.tile([E, GW], F32, tag="x", name=f"vps{c}")
            nc.tensor.matmul(v_ps, wv, mapT[:, sl], start=True, stop=True)
            # glu(v) = a * sigmoid(b); sigmoid(b) = 0.5*tanh(b/2) + 0.5
            th = sb_sm.tile([H, GW], F32, tag="th", name=f"th{c}")
            nc.scalar.activation(th, v_ps[H:E, :], AF.Tanh, scale=0.5)
            nc.vector.tensor_scalar(out=th, in0=th, scalar1=0.5, scalar2=0.5,
                                    op0=ALU.mult, op1=ALU.add)
            nc.vector.tensor_tensor(out=gmT[0:H, sl], in0=v_ps[0:H, :], in1=th,
                                    op=ALU.mult)

        # ---------------- prologue: obs-side projections ----------------
        for c in range(NO // GW):
            sl = slice(c * GW, (c + 1) * GW)
            k_ps = ps_aux.tile([E, GW], F32, tag="x", name=f"okps{c}")
            nc.tensor.matmul(k_ps, wk, obsT[:, sl], start=True, stop=True)
            if c % 2 == 0:
                nc.vector.tensor_copy(okT[:, sl], k_ps)
            else:
                nc.scalar.copy(okT[:, sl], k_ps)

            v_ps = ps_aux.tile([P, 4, E], F32, tag="x", name=f"ovps{c}")
            for b in range(4):
                blk = c * 4 + b
                nc.tensor.matmul(v_ps[:, b, :], obsT[:, blk * P:(blk + 1) * P],
                                 wv, start=True, stop=True)
            tho = sb_sm.tile([P, 4, H], F32, tag="tho", name=f"tho{c}")
            nc.scalar.activation(tho, v_ps[:, :, H:E], AF.Tanh, scale=0.5)
            nc.vector.tensor_scalar(out=tho, in0=tho, scalar1=0.5, scalar2=0.5,
                                    op0=ALU.mult, op1=ALU.add)
            nc.vector.tensor_tensor(out=gob[:, c * 4:(c + 1) * 4, 0:H],
                                    in0=v_ps[:, :, 0:H], in1=tho, op=ALU.mult)

        # ---------------- main loop: attention ----------------
        for hp in range(2):
            agg0 = ps_agg.tile([H + 1, GW], F32, tag="agg", name=f"agg{hp}_0")
            agg1 = ps_agg.tile([H + 1, GW], F32, tag="agg", name=f"agg{hp}_1")
            g0 = 2 * hp
            g1 = 2 * hp + 1
            for ob in range(NOB):
                ko = okT[:, ob * P:(ob + 1) * P]
                go = gob[:, ob, :]
                st = ps_st.tile([P, 2 * GW], F32, tag="st", name=f"st{hp}_{ob}")
                nc.tensor.matmul(st[:, 0:GW], ko, qT[:, g0 * GW:(g0 + 1) * GW],
                                 start=True, stop=True)
                nc.tensor.matmul(st[:, GW:2 * GW], ko, qT[:, g1 * GW:(g1 + 1) * GW],
                                 start=True, stop=True)
                pt = sb_pt.tile([P, 2 * GW], F32R, tag="pt", name=f"pt{hp}_{ob}")
                nc.scalar.activation(pt, st, AF.Exp, scale=1.0 / TEMP)
                nc.tensor.matmul(agg0, go, pt[:, 0:GW],
                                 start=(ob == 0), stop=(ob == NOB - 1))
                nc.tensor.matmul(agg1, go, pt[:, GW:2 * GW],
                                 start=(ob == 0), stop=(ob == NOB - 1))
            for g, agg in ((g0, agg0), (g1, agg1)):
                sl = slice(g * GW, (g + 1) * GW)
                nc.vector.tensor_copy(ags[0:H, sl], agg[0:H, :])
                # denom = obs expsum + selfexp
                nc.vector.tensor_tensor(out=ags[H:H + 1, sl], in0=agg[H:H + 1, :],
                                        in1=gmT[H:H + 1, sl], op=ALU.add)

        # ---------------- epilogue ----------------
        for t in range(NT):
            sl = slice(t * P, (t + 1) * P)
            # [U | denom] row-major via extended Wo (row 32 -> col 64)
            ud = ps_agg.tile([P, E + 2], F32, tag="agg", name=f"ud{t}")
            nc.tensor.matmul(ud, ags[:, sl], woe, start=True, stop=True)
            # [G | selfexp] row-major via the same extended Wo
            g_ps = ps_aux.tile([P, E + 2], F32, tag="x", name=f"gps{t}")
            nc.tensor.matmul(g_ps, gmT[:, sl], woe, start=True, stop=True)
            rden = sb_sm.tile([P, 1], F32, tag="rden", name=f"rden{t}")
            nc.vector.reciprocal(rden, ud[:, E:E + 1])
            gxs = sb_sm.tile([P, E + 2], F32, tag="gxs", name=f"gxs{t}")
            nc.vector.tensor_copy(gxs, g_ps)
            ut = sb_sm.tile([P, E], F32, tag="ut", name=f"ut{t}")
            # numer@Wo + selfexp * (glu(map_v)@Wo)
            nc.vector.scalar_tensor_tensor(out=ut, in0=gxs[:, 0:E],
                                           scalar=gxs[:, E:E + 1],
                                           in1=ud[:, 0:E],
                                           op0=ALU.mult, op1=ALU.add)
            # out_pre = agg@Wo / denom + map + bo
            nc.vector.scalar_tensor_tensor(out=out_pre[:, t, :], in0=ut,
                                           scalar=rden, in1=map_pb[:, t, :],
                                           op0=ALU.mult, op1=ALU.add)
            stats = sb_sm.tile([P, 6], F32, tag="stats", name=f"stats{t}")
            nc.vector.bn_stats(stats, out_pre[:, t, :])
            nc.vector.bn_aggr(mvC[:, t, :], stats)

        # rstd = 1/sqrt(var + eps): ACT sqrt (one table switch, end of
        # kernel) + DVE reciprocal + one float Newton polish.
        vpe = sb_sm.tile([P, NT], F32, tag="vpe", name="vpe")
        nc.vector.tensor_scalar_add(vpe, mvC[:, :, 1], EPS)
        sd = sb_sm.tile([P, NT], F32, tag="sd", name="sd")
        nc.scalar.activation(sd, vpe, AF.Sqrt)
        nc.vector.reciprocal(rstd, sd)
        sc1 = sb_sm.tile([P, NT], F32, tag="sc1", name="nsc1")
        nc.vector.tensor_tensor(out=sc1, in0=rstd, in1=rstd, op=ALU.mult)
        nc.vector.tensor_tensor(out=sc1, in0=sc1, in1=vpe, op=ALU.mult)
        nc.vector.tensor_scalar(out=sc1, in0=sc1, scalar1=-0.5, scalar2=1.5,
                                op0=ALU.mult, op1=ALU.add)
        nc.vector.tensor_tensor(out=rstd, in0=rstd, in1=sc1, op=ALU.mult)

        for t in range(NT):
            xn = sb_sm.tile([P, E], F32, tag="xn", name=f"xn{t}")
            nc.vector.tensor_scalar(out=xn, in0=out_pre[:, t, :],
                                    scalar1=mvC[:, t, 0:1],
                                    scalar2=rstd[:, t:t + 1],
                                    op0=ALU.subtract, op1=ALU.mult)
            nc.vector.tensor_tensor(out=xn, in0=xn, in1=ga_b, op=ALU.mult)
            nc.vector.tensor_tensor(out=out_all[:, t, :], in0=xn, in1=be_b,
                                    op=ALU.add)

        nc.sync.dma_start(out_d.rearrange("(t p) e -> p t e", p=P), out_all)

        if dbg is not None:
            nc.sync.dma_start(dbg["qT"], qT)
            nc.sync.dma_start(dbg["gmT"], gmT)
            nc.sync.dma_start(dbg["ags"], ags)
            nc.sync.dma_start(dbg["okT"], okT)
            nc.sync.dma_start(dbg["gob"], gob.rearrange("p a b -> p (a b)"))
            nc.sync.dma_start(dbg["out_pre"], out_pre.rearrange("p a b -> p (a b)"))
            nc.sync.dma_start(dbg["mvC"], mvC.rearrange("p a b -> p (a b)"))
            nc.sync.dma_start(dbg["rstd"], rstd)


_CACHED = None


def _build(debug=False):
    global _CACHED
    if _CACHED is not None and not debug:
        return _CACHED
    nc = bacc.Bacc("TRN2", target_bir_lowering=False, debug=False)

    def din(name, shape, dt=F32):
        return nc.dram_tensor(name, shape, dt, kind="ExternalInput").ap()

    map_rows_d = din("map_rows", [NS, E])
    mapT_d = din("mapT", [E, NS], F32R)
    obsT_d = din("obsT", [E, NO], F32R)
    wq_d = din("Wq", [E, E], F32R)
    wk_d = din("Wk", [E, E], F32R)
    wv_d = din("Wv", [E, E], F32R)
    woe_d = din("Wo_ext", [H + 1, E + 2], F32R)
    bo_d = din("bo", [E])
    gamma_d = din("gamma", [E])
    beta_d = din("beta", [E])
    ones_d = din("ones_c", [E], F32R)
    out_d = nc.dram_tensor("out", [NS, E], F32, kind="ExternalOutput").ap()

    dbg = None
    if debug:
        def dout(name, shape, dt=F32):
            return nc.dram_tensor(name, shape, dt, kind="ExternalOutput").ap()
        dbg = {
            "qT": dout("dbg_qT", [E, NS], F32R),
            "gmT": dout("dbg_gmT", [H + 1, NS], F32R),
            "ags": dout("dbg_ags", [H + 1, NS], F32R),
            "okT": dout("dbg_okT", [E, NO], F32R),
            "gob": dout("dbg_gob", [P, NOB * (H + 1)], F32R),
            "out_pre": dout("dbg_out_pre", [P, NT * E]),
            "mvC": dout("dbg_mvC", [P, NT * 2]),
            "rstd": dout("dbg_rstd", [P, NT]),
        }

    with tile.TileContext(nc) as tc:
        _emit(tc, out_d, map_rows_d, mapT_d, obsT_d, wq_d, wk_d, wv_d, woe_d,
              bo_d, gamma_d, beta_d, ones_d, dbg=dbg)
    nc.compile()
    if not debug:
        _CACHED = nc
    return nc


def _prep_in_maps(map_code, obs_code, Wq, Wk, Wv, Wo, bo, gamma, beta):
    f = np.float32
    map_code = np.ascontiguousarray(np.asarray(map_code, dtype=f))
    obs_code = np.asarray(obs_code, dtype=f)
    obsT = np.ascontiguousarray(obs_code.T)
    woe = np.zeros((H + 1, E + 2), dtype=f)
    woe[0:H, 0:E] = np.asarray(Wo, dtype=f)
    woe[H, E] = 1.0        # row 32 (denom / selfexp) passes through to col 64
    shared = {
        "obsT": obsT,
        "Wq": np.ascontiguousarray(np.asarray(Wq, dtype=f)),
        "Wk": np.ascontiguousarray(np.asarray(Wk, dtype=f)),
        "Wv": np.ascontiguousarray(np.asarray(Wv, dtype=f)),
        "Wo_ext": woe,
        "bo": np.ascontiguousarray(np.asarray(bo, dtype=f)),
        "gamma": np.ascontiguousarray(np.asarray(gamma, dtype=f)),
        "beta": np.ascontiguousarray(np.asarray(beta, dtype=f)),
        "ones_c": np.ones((E,), dtype=f),
    }
    in_maps = []
    for i in range(NCORES):
        shard = map_code[i * NS:(i + 1) * NS]
        m = dict(shared)
        m["map_rows"] = shard
        m["mapT"] = np.ascontiguousarray(shard.T)
        in_maps.append(m)
    return in_maps


def run(trace=False, **inputs):
    nc = _build()
    in_maps = _prep_in_maps(**inputs)
    res = run_bass_kernel_spmd(nc, in_maps, list(range(NCORES)), trace=trace)
    out = np.concatenate([res.results[i]["out"] for i in range(NCORES)], axis=0)
    return out, res


def kernel(**inputs):
    out, _ = run(trace=False, **inputs)
    return out
